# revision 3
# baseline (speedup 1.0000x reference)
"""FBCritic embedding-lookup kernel for 8 Trainium2 NeuronCores.

Math (reference):
    fwd_idx = clip(obs)*10 + clip(act)            # [8192]
    bwd_idx = clip(fobs)*10 + clip(fact)          # [8192]
    F = W_f[fwd_idx]                              # [8192, 64]
    B = W_b[bwd_idx]                              # [8192, 64]
    out = F @ B.T                                 # [8192, 8192] f32

Sharding: data-parallel over the forward batch. Core c computes output rows
[c*1024, (c+1)*1024). Each core gathers its own 1024 forward rows and all
8192 backward rows from the (replicated) tables with indirect DMA (one
instruction per 128 rows: the HW consumes one index per dest partition and
reads one 256B table row per index), PE-transposes the gathered [128, 64]
tiles into [64, 128] operand layout, then runs f32r matmuls tiled
[128 x 512] and streams [128, 1024] strips to HBM.

The pipeline is column-chunk-outer so matmuls and output DMA for early
column chunks overlap the gathers/transposes of later chunks. Output DMAs
alternate between the two HWDGE queues (sync/scalar); PSUM->SBUF strip
copies are split ~3:1 between vector and scalar engines.
"""

import numpy as np

NUM_OBS = 100000
NUM_ACT = 10
V = NUM_OBS * NUM_ACT  # 1_000_000 table rows
D = 64                 # repr dim
B = 8192               # batch
N_CORES = 8
M = B // N_CORES       # 1024 output rows per core
P = 128                # partitions

_CACHE = {}
TRACE = False
LAST_RESULT = None


def _build_nc():
    import concourse.bass as bass
    import concourse.tile as tile
    from concourse import bacc, mybir
    from concourse.masks import make_identity

    f32 = mybir.dt.float32
    f32r = mybir.dt.float32r
    i32 = mybir.dt.int32

    nc = bacc.Bacc("TRN2", target_bir_lowering=False, debug=False)

    wf = nc.dram_tensor("wf", [V, D], f32, kind="ExternalInput").ap()
    wb = nc.dram_tensor("wb", [V, D], f32, kind="ExternalInput").ap()
    idxf_d = nc.dram_tensor("idxf", [P, M // P], i32, kind="ExternalInput").ap()
    idxb_d = nc.dram_tensor("idxb", [P, B // P], i32, kind="ExternalInput").ap()
    out_d = nc.dram_tensor("out", [M, B], f32, kind="ExternalOutput").ap()

    GF = M // P     # 8 forward 128-row groups
    GB = B // P     # 64 backward 128-row groups
    NJ = 512        # matmul moving free dim (one PSUM bank)
    JP = 1024       # output strip width (4KB DMA descriptors)
    NPAIR = B // JP

    n_dma = [0]
    n_copy = [0]

    def out_dma_start(dst, src):
        e = nc.sync if n_dma[0] % 2 == 0 else nc.scalar
        n_dma[0] += 1
        e.dma_start(dst, src)

    def strip_copy(dst, src):
        if n_copy[0] % 4 == 3:
            nc.scalar.copy(out=dst, in_=src)
        else:
            nc.vector.tensor_copy(out=dst, in_=src)
        n_copy[0] += 1

    def gather128(pool, table, idx_tile, g):
        t = pool.tile([P, D], f32, tag="bg")
        nc.gpsimd.indirect_dma_start(
            out=t[:],
            out_offset=None,
            in_=table[:],
            in_offset=bass.IndirectOffsetOnAxis(ap=idx_tile[:, g:g + 1], axis=0),
        )
        return t

    with tile.TileContext(nc) as tc:
        with (
            tc.tile_pool(name="const", bufs=1) as const_pool,
            tc.tile_pool(name="idx", bufs=1) as idx_pool,
            tc.tile_pool(name="bg", bufs=16) as bg_pool,
            tc.tile_pool(name="ops", bufs=1) as ops_pool,
            tc.tile_pool(name="strip", bufs=10) as strip_pool,
            tc.tile_pool(name="tpsum", bufs=2, space="PSUM") as tpsum_pool,
            tc.tile_pool(name="mpsum", bufs=3, space="PSUM") as mpsum_pool,
        ):
            identity = const_pool.tile([P, P], f32)
            make_identity(nc, identity[:])

            idxf = idx_pool.tile([P, GF], i32, tag="idxf")
            idxb = idx_pool.tile([P, GB], i32, tag="idxb")
            nc.sync.dma_start(idxf[:], idxf_d[:])
            nc.sync.dma_start(idxb[:], idxb_d[:])

            # Forward operand: gather 8x128 rows, transpose to [64, 1024] f32r.
            fwdT = ops_pool.tile([D, M], f32r, tag="fwdT")
            for q in range(GF // 4):
                pt = tpsum_pool.tile([D, 512], f32, tag="pt")
                for r in range(4):
                    t = gather128(bg_pool, wf, idxf, q * 4 + r)
                    nc.tensor.transpose(
                        out=pt[:, r * P:(r + 1) * P], in_=t[:], identity=identity[:]
                    )
                nc.vector.tensor_copy(out=fwdT[:, q * 512:(q + 1) * 512], in_=pt[:])

            # Column-chunk-outer pipeline over the backward reprs.
            for jp in range(NPAIR):
                bt = ops_pool.tile([D, JP], f32r, tag=f"bwdT{jp}")
                for h in range(2):
                    pt = tpsum_pool.tile([D, 512], f32, tag="pt")
                    for r in range(4):
                        g = jp * 8 + h * 4 + r
                        t = gather128(bg_pool, wb, idxb, g)
                        nc.tensor.transpose(
                            out=pt[:, r * P:(r + 1) * P],
                            in_=t[:],
                            identity=identity[:],
                        )
                    nc.vector.tensor_copy(
                        out=bt[:, h * 512:(h + 1) * 512], in_=pt[:]
                    )

                for i in range(M // P):  # 8 row tiles
                    strip = strip_pool.tile([P, JP], f32, tag="strip")
                    ps = mpsum_pool.tile([P, JP], f32, tag="ps")  # 2 banks
                    for h in range(2):
                        nc.tensor.matmul(
                            out=ps[:, h * NJ:(h + 1) * NJ],
                            lhsT=fwdT[:, i * P:(i + 1) * P],
                            rhs=bt[:, h * NJ:(h + 1) * NJ],
                            start=True,
                            stop=True,
                        )
                    strip_copy(strip[:], ps[:])  # one [128, 1024] copy
                    out_dma_start(
                        out_d[i * P:(i + 1) * P, jp * JP:(jp + 1) * JP], strip[:]
                    )

    nc.compile()
    return nc


def _get_nc():
    if "nc" not in _CACHE:
        _CACHE["nc"] = _build_nc()
    return _CACHE["nc"]


def _ravel_clip(obs, act):
    o = np.clip(obs.astype(np.int64), 0, NUM_OBS - 1)
    a = np.clip(act.astype(np.int64), 0, NUM_ACT - 1)
    return (o * NUM_ACT + a).astype(np.int32)


def make_in_maps(observations, actions, future_observations, future_actions,
                 W_f, W_b):
    fwd_idx = _ravel_clip(np.asarray(observations), np.asarray(actions))
    bwd_idx = _ravel_clip(np.asarray(future_observations),
                          np.asarray(future_actions))
    wf = np.ascontiguousarray(np.asarray(W_f, dtype=np.float32))
    wb = np.ascontiguousarray(np.asarray(W_b, dtype=np.float32))
    # [p, g] = idx[g*128 + p]
    idxb = np.ascontiguousarray(bwd_idx.reshape(B // P, P).T)
    in_maps = []
    for c in range(N_CORES):
        idxf = np.ascontiguousarray(
            fwd_idx[c * M:(c + 1) * M].reshape(M // P, P).T
        )
        in_maps.append({"wf": wf, "wb": wb, "idxf": idxf, "idxb": idxb})
    return in_maps


def kernel(**inputs):
    from concourse.bass_utils import run_bass_kernel_spmd

    in_maps = make_in_maps(
        inputs["observations"], inputs["actions"],
        inputs["future_observations"], inputs["future_actions"],
        inputs["W_f"], inputs["W_b"],
    )
    res = run_bass_kernel_spmd(_get_nc(), in_maps, core_ids=list(range(N_CORES)),
                               trace=TRACE)
    globals()["LAST_RESULT"] = res
    return np.concatenate(
        [res.results[c]["out"] for c in range(N_CORES)], axis=0
    )

